# revision 37
# baseline (speedup 1.0000x reference)
"""Trainium2 Bass kernel for the differentiable LogicLayer forward pass.

Math (per output neuron j with a = x[:, idx_a[j]], b = x[:, idx_b[j]]):
    w      = softmax(weights[j])          # [14]
    coeffs = w @ OP_COEFFS                # [4] -> c0, ca, cb, cab
    out[:, j] = c0 + ca*a + cb*b + cab*a*b

Sharding: data-parallel over batch across 8 NeuronCores (1024 rows each);
weights / indices replicated.  Per core the kernel works feature-major:
partition p holds output neuron pi[t*128 + p] where pi sorts neurons by
idx_a; the free dim holds the 1024-sample batch shard, all bulk data fp16.

The a-side gather is done by the Tensor engine: the batch shard xT
[4096, 1024] lives in SBUF as 32 blocks of 128 rows, and each group of
128 pi-consecutive neurons pulls its a-rows with one-hot stationary
matmuls (sorted neurons touch 1-2 blocks, so ~160 matmuls total,
accumulated in PSUM across block boundaries).  The psum evacuation fuses
the per-neuron affine u = cab*a + cb (ACT scale/bias, or DVE fused
tensor_scalar for a balanced split).  The b-side gather stays on the
SWDGE dma_gather with pi-permuted indices.  Output is stored fp16 in pi
order and unpermuted/widened on the host.
"""

import sys

import numpy as np

try:  # the axon sitecustomize usually provides concourse already
    import concourse  # noqa: F401
except ImportError:  # pragma: no cover
    sys.path.insert(0, "/opt/trn_rl_repo")

import concourse.bacc as bacc
import concourse.mybir as mybir
import concourse.tile as tile
from concourse.bass import MemorySpace
from concourse.bass_utils import run_bass_kernel_spmd
from concourse.library_config import mlp as mlp_library

F32 = mybir.dt.float32
F16 = mybir.dt.float16
I16 = mybir.dt.int16

NCORES = 8
BATCH, IN_DIM, OUT_DIM, NOPS = 8192, 4096, 16384, 14
B = BATCH // NCORES            # 1024 batch rows per core
NJC = 512                      # output neurons per gather chunk
NCH = OUT_DIM // NJC           # 32 chunks
SL = NJC // 128                # 4 partition-slices (groups) per chunk
NT = OUT_DIM // 128            # 128 neuron groups
NBLK = IN_DIM // 128           # 32 xT row-blocks resident in SBUF

_OP_COEFFS = np.array([
    [0,  0,  0,  1],
    [0,  1,  0, -1],
    [0,  1,  0,  0],
    [0,  0,  1, -1],
    [0,  0,  1,  0],
    [0,  1,  1, -2],
    [0,  1,  1, -1],
    [1, -1, -1,  1],
    [1, -1, -1,  2],
    [1,  0, -1,  0],
    [1,  0, -1,  1],
    [1, -1,  0,  0],
    [1, -1,  0,  1],
    [1,  0,  0, -1],
], dtype=np.float32)

# fraction of groups whose v-affine runs on ACT (rest fused on DVE),
# balancing ACT (u always + lam*v) against DVE (v + the two tensor_tensor)
_ACT_HEAVY_NUM, _ACT_HEAVY_DEN = 17, 32


def build_program(plan):
    """Build + compile the per-core Bass program.

    plan[g] = tuple of xT 128-row blocks the g-th sorted neuron group
    draws its a-rows from (usually 1 block, 2 at a sort boundary).
    """
    nmm = sum(len(bs) for bs in plan)

    nc = bacc.Bacc("TRN2", target_bir_lowering=False, debug=False,
                   num_devices=NCORES)

    xt = nc.dram_tensor("xt", [IN_DIM, B], F16, kind="ExternalInput")
    # p-major duplicate of xt so the SBUF load is one big contiguous
    # descriptor per partition (upload happens before the kernel runs)
    xtp = nc.dram_tensor("xtp", [128, NBLK, B], F16, kind="ExternalInput")
    oh = nc.dram_tensor("oh", [128, nmm, 128], F16, kind="ExternalInput")
    wre = nc.dram_tensor("wre", [128, NT, NOPS], F32, kind="ExternalInput")
    opc = nc.dram_tensor("opc", [128, 4, NOPS], F32, kind="ExternalInput")
    idxb = nc.dram_tensor("idxb", [128, OUT_DIM // 16], I16, kind="ExternalInput")
    # p-major output: [p, t, b] holds neuron pi[t*128+p] -> 8 KiB
    # contiguous per partition per chunk store
    out = nc.dram_tensor("out", [128, NT, B], F16, kind="ExternalOutput")

    out_r = out.ap()

    mult = mybir.AluOpType.mult
    add = mybir.AluOpType.add
    ident = mybir.ActivationFunctionType.Identity
    expf = mybir.ActivationFunctionType.Exp

    with tile.TileContext(nc) as tc:
        nc.gpsimd.load_library(mlp_library)
        with (
            tc.tile_pool(name="const", bufs=1) as cpool,
            tc.tile_pool(name="coef", bufs=1) as kpool,
        ):
            # small inputs first: the coef prologue and the first b-gather
            # must not queue behind the big resident loads
            ib_sb = cpool.tile([128, OUT_DIM // 16], I16)
            nc.sync.dma_start(ib_sb[:], idxb.ap())

            # ---- coefficients: softmax over the 14 ops, collapsed to 4 ----
            with tc.tile_pool(name="init", bufs=1) as ipool:
                w_sb = ipool.tile([128, NT, NOPS], F32)
                nc.sync.dma_start(w_sb[:], wre.ap())
                opc_sb = ipool.tile([128, 4, NOPS], F32)
                nc.sync.dma_start(opc_sb[:], opc.ap())

                # resident tiles; pieces stream in during the first chunks
                xts = cpool.tile([128, NBLK, B], F16)
                oh_sb = cpool.tile([128, nmm, 128], F16)
                qb = NBLK // 8
                qm = (nmm + 7) // 8

                # piece 0 of both first: the first matmuls need xts block 0
                # AND the first stationaries, the rest can trickle in
                for q in (0, 1):
                    nc.sync.dma_start(xts[:, q * qb:(q + 1) * qb],
                                      xtp.ap()[:, q * qb:(q + 1) * qb])
                    nc.sync.dma_start(oh_sb[:, q * qm:(q + 1) * qm],
                                      oh.ap()[:, q * qm:(q + 1) * qm])
                for q in range(2, 8):
                    nc.sync.dma_start(xts[:, q * qb:(q + 1) * qb],
                                      xtp.ap()[:, q * qb:(q + 1) * qb])
                for q in range(2, 8):
                    lo, hi = q * qm, min((q + 1) * qm, nmm)
                    if lo < hi:
                        nc.sync.dma_start(oh_sb[:, lo:hi], oh.ap()[:, lo:hi])

                e_sb = ipool.tile([128, NT, NOPS], F32)
                nc.scalar.activation(e_sb[:], w_sb[:], expf)
                ssum = ipool.tile([128, NT], F32)
                nc.vector.tensor_reduce(ssum[:], e_sb[:],
                                        mybir.AxisListType.X, add)
                rsum = ipool.tile([128, NT], F32)
                nc.vector.reciprocal(rsum[:], ssum[:])

                # coef[m]: [128, NT] with (p, t) = coeff_m[pi[t*128+p]];
                # cab/cb first (the always-on-ACT u-evac needs them)
                coef = {}
                for m in (3, 2, 1, 0):
                    tmp = ipool.tile([128, NT, NOPS], F32, tag="ctmp",
                                     name="ctmp")
                    nc.vector.tensor_tensor(
                        tmp[:], e_sb[:],
                        opc_sb[:, m][:, None, :].broadcast_to(
                            [128, NT, NOPS]),
                        op=mult)
                    cs = ipool.tile([128, NT], F32, tag="csum", name="csum")
                    nc.vector.tensor_reduce(cs[:], tmp[:],
                                            mybir.AxisListType.X, add)
                    cm = kpool.tile([128, NT], F32, tag=f"coef{m}",
                                    name=f"coef{m}")
                    nc.vector.tensor_tensor(cm[:], cs[:], rsum[:], op=mult)
                    coef[m] = cm
                c0, ca, cb, cab = (coef[m] for m in range(4))

            # ---- main loop: PE a-gather, SWDGE b-gather, combine, store ----
            with (
                tc.tile_pool(name="gb", bufs=3) as bpool,
                tc.tile_pool(name="go", bufs=2) as opool,
                tc.tile_pool(name="uv", bufs=2) as uvpool,
                tc.tile_pool(name="ps", bufs=4,
                             space=MemorySpace.PSUM) as ppool,
            ):
                w16 = NJC // 16  # idx columns per chunk
                mi = 0           # running matmul index into oh
                act_acc = 0      # lam-split accumulator
                for ci in range(NCH):
                    bt = bpool.tile([128, SL, B], F16)
                    nc.gpsimd.dma_gather(
                        bt[:], xt.ap(), ib_sb[:, ci * w16:(ci + 1) * w16],
                        NJC, NJC, B)
                    ot = opool.tile([128, SL, B], F16)
                    u = uvpool.tile([128, SL, B], F16, tag="u")
                    v = uvpool.tile([128, SL, B], F16, tag="v")
                    for s in range(SL):
                        g = ci * SL + s
                        blocks = plan[g]
                        pt = ppool.tile([128, B], F32, tag="ps")
                        for k, c in enumerate(blocks):
                            # PSUM bank limit: N <= 512 fp32 per matmul
                            for h in range(2):
                                hb = h * (B // 2)
                                nc.tensor.matmul(
                                    pt[:, hb:hb + B // 2], oh_sb[:, mi],
                                    xts[:, c, hb:hb + B // 2],
                                    start=(k == 0),
                                    stop=(k == len(blocks) - 1))
                            mi += 1
                        # u = cab*a + cb  (ACT evac of psum, fused affine)
                        nc.scalar.activation(u[:, s], pt[:], ident,
                                             bias=cb[:, g:g + 1],
                                             scale=cab[:, g:g + 1])
                        # v = ca*a + c0  (ACT or DVE, balanced split)
                        act_acc += _ACT_HEAVY_NUM
                        if act_acc >= _ACT_HEAVY_DEN:
                            act_acc -= _ACT_HEAVY_DEN
                            nc.scalar.activation(v[:, s], pt[:], ident,
                                                 bias=c0[:, g:g + 1],
                                                 scale=ca[:, g:g + 1])
                        else:
                            nc.vector.tensor_scalar(
                                v[:, s], pt[:], ca[:, g:g + 1],
                                c0[:, g:g + 1], op0=mult, op1=add)
                    # out = u*b + v over the whole chunk (DVE, fp16 2x)
                    nc.vector.tensor_tensor(u[:], u[:], bt[:], op=mult)
                    nc.vector.tensor_tensor(ot[:], u[:], v[:], op=add)
                    nc.sync.dma_start(out_r[:, ci * SL:(ci + 1) * SL], ot[:])
                assert mi == nmm

    nc.compile()
    return nc


_PROGRAMS = {}
_NEEDS_INPUTS = True


def _make_plan(idx_a):
    """Sorted-neuron permutation + per-group xT block lists."""
    pi = np.argsort(np.asarray(idx_a), kind="stable")
    ia_s = np.asarray(idx_a)[pi].astype(np.int64)
    plan = []
    for g in range(NT):
        rows = ia_s[g * 128:(g + 1) * 128]
        plan.append(tuple(sorted(set(int(r) // 128 for r in rows))))
    return pi, ia_s, tuple(plan)


def _get_program(x=None, weights=None, idx_a=None, idx_b=None):
    _, _, plan = _make_plan(idx_a)
    if plan not in _PROGRAMS:
        _PROGRAMS[plan] = build_program(plan)
    return _PROGRAMS[plan]


def _wrap_idx(idx):
    """[OUT_DIM] int -> SWDGE-wrapped int16 [128, OUT_DIM//16].

    Per NJC-chunk c, columns [c*NJC//16:(c+1)*NJC//16] hold that chunk's
    indices with index i at (partition i%16, column i//16), replicated
    across the 8 groups of 16 partitions (one per Q7 core).
    """
    i16 = idx.astype(np.int16).reshape(NCH, NJC // 16, 16)
    w = i16.transpose(2, 0, 1).reshape(16, NCH * (NJC // 16))
    return np.ascontiguousarray(np.tile(w, (8, 1)))


def _build_oh(ia_s, plan):
    """One-hot stationaries [128, nmm, 128]: column m of matmul (g, c)
    selects xT row ia_s[g*128+m] when it lies in block c, else zero."""
    nmm = sum(len(bs) for bs in plan)
    oh = np.zeros((128, nmm, 128), dtype=np.float16)
    mi = 0
    cols = np.arange(128)
    for g in range(NT):
        rows = ia_s[g * 128:(g + 1) * 128]
        for c in plan[g]:
            rel = rows - 128 * c
            m = (rel >= 0) & (rel < 128)
            oh[rel[m], mi, cols[m]] = 1.0
            mi += 1
    return oh


def prepare_in_maps(x, weights, idx_a, idx_b):
    x = np.asarray(x, dtype=np.float32)
    weights = np.asarray(weights, dtype=np.float32)
    idx_a = np.asarray(idx_a)
    idx_b = np.asarray(idx_b)

    pi, ia_s, plan = _make_plan(idx_a)
    oh = _build_oh(ia_s, plan)
    wre = np.ascontiguousarray(
        weights[pi].reshape(NT, 128, NOPS).transpose(1, 0, 2))
    opc = np.ascontiguousarray(
        np.broadcast_to(_OP_COEFFS.T[None, :, :],
                        (128, 4, NOPS))).astype(np.float32)
    ib = _wrap_idx(idx_b[pi])

    global _PI
    _PI = pi
    x16 = x.astype(np.float16)
    in_maps = []
    for c in range(NCORES):
        xt = np.ascontiguousarray(x16[c * B:(c + 1) * B].T)
        xtp = np.ascontiguousarray(
            xt.reshape(NBLK, 128, B).transpose(1, 0, 2))
        in_maps.append({"xt": xt, "xtp": xtp, "oh": oh, "wre": wre,
                        "opc": opc, "idxb": ib})
    return in_maps


_PI = None


def assemble_output(results):
    out = np.empty((BATCH, OUT_DIM), dtype=np.float32)
    for c in range(NCORES):
        dev = results[c]["out"]  # [128, NT, B], neuron pi[t*128+p]
        out[c * B:(c + 1) * B, _PI] = \
            dev.transpose(2, 1, 0).reshape(B, OUT_DIM)
    return out


def kernel(x, weights, idx_a, idx_b):
    nc = _get_program(idx_a=idx_a)
    in_maps = prepare_in_maps(x, weights, idx_a, idx_b)
    res = run_bass_kernel_spmd(nc, in_maps, list(range(NCORES)))
    return assemble_output(res.results)


# revision 38
# speedup vs baseline: 1.0152x; 1.0152x over previous
"""Trainium2 Bass kernel for the differentiable LogicLayer forward pass.

Math (per output neuron j with a = x[:, idx_a[j]], b = x[:, idx_b[j]]):
    w      = softmax(weights[j])          # [14]
    coeffs = w @ OP_COEFFS                # [4] -> c0, ca, cb, cab
    out[:, j] = c0 + ca*a + cb*b + cab*a*b

Sharding: data-parallel over batch across 8 NeuronCores (1024 rows each);
weights / indices replicated.  Per core the kernel works feature-major:
partition p holds output neuron pi[t*128 + p] where pi sorts neurons by
idx_a; the free dim holds the 1024-sample batch shard, all bulk data fp16.

The a-side gather is done by the Tensor engine: the batch shard xT
[4096, 1024] lives in SBUF as 32 blocks of 128 rows, and each group of
128 pi-consecutive neurons pulls its a-rows with one-hot stationary
matmuls (sorted neurons touch 1-2 blocks, so ~160 matmuls total,
accumulated in PSUM across block boundaries).  The psum evacuation fuses
the per-neuron affine u = cab*a + cb (ACT scale/bias, or DVE fused
tensor_scalar for a balanced split).  The b-side gather stays on the
SWDGE dma_gather with pi-permuted indices.  Output is stored fp16 in pi
order and unpermuted/widened on the host.
"""

import sys

import numpy as np

try:  # the axon sitecustomize usually provides concourse already
    import concourse  # noqa: F401
except ImportError:  # pragma: no cover
    sys.path.insert(0, "/opt/trn_rl_repo")

import concourse.bacc as bacc
import concourse.mybir as mybir
import concourse.tile as tile
from concourse.bass import MemorySpace
from concourse.bass_utils import run_bass_kernel_spmd
from concourse.library_config import mlp as mlp_library

F32 = mybir.dt.float32
F16 = mybir.dt.float16
I16 = mybir.dt.int16

NCORES = 8
BATCH, IN_DIM, OUT_DIM, NOPS = 8192, 4096, 16384, 14
B = BATCH // NCORES            # 1024 batch rows per core
NJC = 512                      # output neurons per gather chunk
NCH = OUT_DIM // NJC           # 32 chunks
SL = NJC // 128                # 4 partition-slices (groups) per chunk
NT = OUT_DIM // 128            # 128 neuron groups
NBLK = IN_DIM // 128           # 32 xT row-blocks resident in SBUF

_OP_COEFFS = np.array([
    [0,  0,  0,  1],
    [0,  1,  0, -1],
    [0,  1,  0,  0],
    [0,  0,  1, -1],
    [0,  0,  1,  0],
    [0,  1,  1, -2],
    [0,  1,  1, -1],
    [1, -1, -1,  1],
    [1, -1, -1,  2],
    [1,  0, -1,  0],
    [1,  0, -1,  1],
    [1, -1,  0,  0],
    [1, -1,  0,  1],
    [1,  0,  0, -1],
], dtype=np.float32)

# fraction of groups whose v-affine runs on ACT (rest fused on DVE),
# balancing ACT (u always + lam*v) against DVE (v + the two tensor_tensor)
_ACT_HEAVY_NUM, _ACT_HEAVY_DEN = 17, 32


def build_program(plan):
    """Build + compile the per-core Bass program.

    plan[g] = tuple of xT 128-row blocks the g-th sorted neuron group
    draws its a-rows from (usually 1 block, 2 at a sort boundary).
    """
    nmm = sum(len(bs) for bs in plan)

    nc = bacc.Bacc("TRN2", target_bir_lowering=False, debug=False,
                   num_devices=NCORES)

    xt = nc.dram_tensor("xt", [IN_DIM, B], F16, kind="ExternalInput")
    # p-major duplicate of xt so the SBUF load is one big contiguous
    # descriptor per partition (upload happens before the kernel runs)
    xtp = nc.dram_tensor("xtp", [128, NBLK, B], F16, kind="ExternalInput")
    oh = nc.dram_tensor("oh", [128, nmm, 128], F16, kind="ExternalInput")
    wre = nc.dram_tensor("wre", [128, NT, NOPS], F32, kind="ExternalInput")
    opc = nc.dram_tensor("opc", [128, 4, NOPS], F32, kind="ExternalInput")
    idxb = nc.dram_tensor("idxb", [128, OUT_DIM // 16], I16, kind="ExternalInput")
    # p-major output: [p, t, b] holds neuron pi[t*128+p] -> 8 KiB
    # contiguous per partition per chunk store
    out = nc.dram_tensor("out", [128, NT, B], F16, kind="ExternalOutput")

    out_r = out.ap()

    mult = mybir.AluOpType.mult
    add = mybir.AluOpType.add
    ident = mybir.ActivationFunctionType.Identity
    expf = mybir.ActivationFunctionType.Exp

    with tile.TileContext(nc) as tc:
        nc.gpsimd.load_library(mlp_library)
        with (
            tc.tile_pool(name="const", bufs=1) as cpool,
            tc.tile_pool(name="coef", bufs=1) as kpool,
        ):
            # small inputs first: the coef prologue and the first b-gather
            # must not queue behind the big resident loads
            ib_sb = cpool.tile([128, OUT_DIM // 16], I16)
            nc.sync.dma_start(ib_sb[:], idxb.ap())

            # ---- coefficients: softmax over the 14 ops, collapsed to 4 ----
            with tc.tile_pool(name="init", bufs=1) as ipool:
                w_sb = ipool.tile([128, NT, NOPS], F32)
                nc.sync.dma_start(w_sb[:], wre.ap())
                opc_sb = ipool.tile([128, 4, NOPS], F32)
                nc.sync.dma_start(opc_sb[:], opc.ap())

                # resident tiles; pieces stream in during the first chunks
                xts = cpool.tile([128, NBLK, B], F16)
                oh_sb = cpool.tile([128, nmm, 128], F16)
                qb = NBLK // 8
                qm = (nmm + 7) // 8

                for q in range(8):
                    nc.sync.dma_start(xts[:, q * qb:(q + 1) * qb],
                                      xtp.ap()[:, q * qb:(q + 1) * qb])
                for q in range(8):
                    lo, hi = q * qm, min((q + 1) * qm, nmm)
                    if lo < hi:
                        nc.sync.dma_start(oh_sb[:, lo:hi], oh.ap()[:, lo:hi])

                e_sb = ipool.tile([128, NT, NOPS], F32)
                nc.scalar.activation(e_sb[:], w_sb[:], expf)
                ssum = ipool.tile([128, NT], F32)
                nc.vector.tensor_reduce(ssum[:], e_sb[:],
                                        mybir.AxisListType.X, add)
                rsum = ipool.tile([128, NT], F32)
                nc.vector.reciprocal(rsum[:], ssum[:])

                # coef[m]: [128, NT] with (p, t) = coeff_m[pi[t*128+p]];
                # cab/cb first (the always-on-ACT u-evac needs them)
                coef = {}
                for m in (3, 2, 1, 0):
                    tmp = ipool.tile([128, NT, NOPS], F32, tag="ctmp",
                                     name="ctmp")
                    nc.vector.tensor_tensor(
                        tmp[:], e_sb[:],
                        opc_sb[:, m][:, None, :].broadcast_to(
                            [128, NT, NOPS]),
                        op=mult)
                    cs = ipool.tile([128, NT], F32, tag="csum", name="csum")
                    nc.vector.tensor_reduce(cs[:], tmp[:],
                                            mybir.AxisListType.X, add)
                    cm = kpool.tile([128, NT], F32, tag=f"coef{m}",
                                    name=f"coef{m}")
                    nc.vector.tensor_tensor(cm[:], cs[:], rsum[:], op=mult)
                    coef[m] = cm
                c0, ca, cb, cab = (coef[m] for m in range(4))

            # ---- main loop: PE a-gather, SWDGE b-gather, combine, store ----
            with (
                tc.tile_pool(name="gb", bufs=3) as bpool,
                tc.tile_pool(name="go", bufs=2) as opool,
                tc.tile_pool(name="uv", bufs=2) as uvpool,
                tc.tile_pool(name="ps", bufs=4,
                             space=MemorySpace.PSUM) as ppool,
            ):
                w16 = NJC // 16  # idx columns per chunk
                mi = 0           # running matmul index into oh
                act_acc = 0      # lam-split accumulator
                for ci in range(NCH):
                    bt = bpool.tile([128, SL, B], F16)
                    nc.gpsimd.dma_gather(
                        bt[:], xt.ap(), ib_sb[:, ci * w16:(ci + 1) * w16],
                        NJC, NJC, B)
                    ot = opool.tile([128, SL, B], F16)
                    u = uvpool.tile([128, SL, B], F16, tag="u")
                    v = uvpool.tile([128, SL, B], F16, tag="v")
                    for s in range(SL):
                        g = ci * SL + s
                        blocks = plan[g]
                        pt = ppool.tile([128, B], F32, tag="ps")
                        for k, c in enumerate(blocks):
                            # PSUM bank limit: N <= 512 fp32 per matmul
                            for h in range(2):
                                hb = h * (B // 2)
                                nc.tensor.matmul(
                                    pt[:, hb:hb + B // 2], oh_sb[:, mi],
                                    xts[:, c, hb:hb + B // 2],
                                    start=(k == 0),
                                    stop=(k == len(blocks) - 1))
                            mi += 1
                        # u = cab*a + cb  (ACT evac of psum, fused affine)
                        nc.scalar.activation(u[:, s], pt[:], ident,
                                             bias=cb[:, g:g + 1],
                                             scale=cab[:, g:g + 1])
                        # v = ca*a + c0  (ACT or DVE, balanced split)
                        act_acc += _ACT_HEAVY_NUM
                        if act_acc >= _ACT_HEAVY_DEN:
                            act_acc -= _ACT_HEAVY_DEN
                            nc.scalar.activation(v[:, s], pt[:], ident,
                                                 bias=c0[:, g:g + 1],
                                                 scale=ca[:, g:g + 1])
                        else:
                            nc.vector.tensor_scalar(
                                v[:, s], pt[:], ca[:, g:g + 1],
                                c0[:, g:g + 1], op0=mult, op1=add)
                    # out = u*b + v over the whole chunk (DVE, fp16 2x)
                    nc.vector.tensor_tensor(u[:], u[:], bt[:], op=mult)
                    nc.vector.tensor_tensor(ot[:], u[:], v[:], op=add)
                    nc.sync.dma_start(out_r[:, ci * SL:(ci + 1) * SL], ot[:])
                assert mi == nmm

    nc.compile()
    return nc


_PROGRAMS = {}
_NEEDS_INPUTS = True


def _make_plan(idx_a):
    """Sorted-neuron permutation + per-group xT block lists."""
    pi = np.argsort(np.asarray(idx_a), kind="stable")
    ia_s = np.asarray(idx_a)[pi].astype(np.int64)
    plan = []
    for g in range(NT):
        rows = ia_s[g * 128:(g + 1) * 128]
        plan.append(tuple(sorted(set(int(r) // 128 for r in rows))))
    return pi, ia_s, tuple(plan)


def _get_program(x=None, weights=None, idx_a=None, idx_b=None):
    _, _, plan = _make_plan(idx_a)
    if plan not in _PROGRAMS:
        _PROGRAMS[plan] = build_program(plan)
    return _PROGRAMS[plan]


def _wrap_idx(idx):
    """[OUT_DIM] int -> SWDGE-wrapped int16 [128, OUT_DIM//16].

    Per NJC-chunk c, columns [c*NJC//16:(c+1)*NJC//16] hold that chunk's
    indices with index i at (partition i%16, column i//16), replicated
    across the 8 groups of 16 partitions (one per Q7 core).
    """
    i16 = idx.astype(np.int16).reshape(NCH, NJC // 16, 16)
    w = i16.transpose(2, 0, 1).reshape(16, NCH * (NJC // 16))
    return np.ascontiguousarray(np.tile(w, (8, 1)))


def _build_oh(ia_s, plan):
    """One-hot stationaries [128, nmm, 128]: column m of matmul (g, c)
    selects xT row ia_s[g*128+m] when it lies in block c, else zero."""
    nmm = sum(len(bs) for bs in plan)
    oh = np.zeros((128, nmm, 128), dtype=np.float16)
    mi = 0
    cols = np.arange(128)
    for g in range(NT):
        rows = ia_s[g * 128:(g + 1) * 128]
        for c in plan[g]:
            rel = rows - 128 * c
            m = (rel >= 0) & (rel < 128)
            oh[rel[m], mi, cols[m]] = 1.0
            mi += 1
    return oh


def prepare_in_maps(x, weights, idx_a, idx_b):
    x = np.asarray(x, dtype=np.float32)
    weights = np.asarray(weights, dtype=np.float32)
    idx_a = np.asarray(idx_a)
    idx_b = np.asarray(idx_b)

    pi, ia_s, plan = _make_plan(idx_a)
    oh = _build_oh(ia_s, plan)
    wre = np.ascontiguousarray(
        weights[pi].reshape(NT, 128, NOPS).transpose(1, 0, 2))
    opc = np.ascontiguousarray(
        np.broadcast_to(_OP_COEFFS.T[None, :, :],
                        (128, 4, NOPS))).astype(np.float32)
    ib = _wrap_idx(idx_b[pi])

    global _PI
    _PI = pi
    x16 = x.astype(np.float16)
    in_maps = []
    for c in range(NCORES):
        xt = np.ascontiguousarray(x16[c * B:(c + 1) * B].T)
        xtp = np.ascontiguousarray(
            xt.reshape(NBLK, 128, B).transpose(1, 0, 2))
        in_maps.append({"xt": xt, "xtp": xtp, "oh": oh, "wre": wre,
                        "opc": opc, "idxb": ib})
    return in_maps


_PI = None


def assemble_output(results):
    out = np.empty((BATCH, OUT_DIM), dtype=np.float32)
    for c in range(NCORES):
        dev = results[c]["out"]  # [128, NT, B], neuron pi[t*128+p]
        out[c * B:(c + 1) * B, _PI] = \
            dev.transpose(2, 1, 0).reshape(B, OUT_DIM)
    return out


def kernel(x, weights, idx_a, idx_b):
    nc = _get_program(idx_a=idx_a)
    in_maps = prepare_in_maps(x, weights, idx_a, idx_b)
    res = run_bass_kernel_spmd(nc, in_maps, list(range(NCORES)))
    return assemble_output(res.results)
